# revision 2
# baseline (speedup 1.0000x reference)
"""Trainium2 Bass kernel for grouped expert GEMM (MoE forward).

Computes out[n, e, d] = sum_k x[n, k] * W[e, k, d] + b[e, d] for
N=16384 tokens, E=64 experts, D=128, fp32.

Expert-parallel across 8 NeuronCores: core m owns experts [8m, 8m+8),
reads the full token matrix, and writes out[:, 8m:8m+8, :].

Per-core layout:
  - host pre-transposes x -> xT [D=128, N] so the contraction dim (D)
    lands on SBUF partitions.
  - the 8 owned experts' weights are concatenated along the free dim:
    wcat [128, 8*128]; psum[t, (e,d)] = xT_blk.T @ wcat.
  - bias is host-broadcast to [128, 8*128] and fused into the
    PSUM->SBUF copy on the vector engine.
  - each 128-token block stores one contiguous 512 KB region of the
    per-core output [N, 8, 128].
"""

import os
import sys

if not any("trn_rl_repo" in p for p in sys.path):
    sys.path.insert(0, "/opt/trn_rl_repo")

from contextlib import ExitStack

import numpy as np

import concourse.bacc as bacc
import concourse.tile as tile
from concourse import mybir
from concourse.bass_utils import run_bass_kernel_spmd

N, E, D = 16384, 64, 128
M = 8                 # cores
EPC = E // M          # experts per core
FREE = EPC * D        # concatenated expert free dim = 1024
MM_N = 512            # max fp32 matmul free dim (one PSUM bank)
NB = N // 128         # 128-token blocks
NCHUNK = 8            # xT load chunks
CH = N // NCHUNK

F32 = mybir.dt.float32
# float32r streams 1 col/cycle (vs 4 for float32) at free dim >= 256.
MM_DT = mybir.dt.float32r if os.environ.get("KERNEL_MM_DT", "f32r") == "f32r" else F32

_built = {}


def _body(nc, xT_d, w_d, b_d, out_v, ctx, tc):
    xpool = ctx.enter_context(tc.tile_pool(name="x", bufs=1))
    cpool = ctx.enter_context(tc.tile_pool(name="const", bufs=1))
    spool = ctx.enter_context(tc.tile_pool(name="stage", bufs=6))
    ppool = ctx.enter_context(tc.tile_pool(name="psum", bufs=6, space="PSUM"))

    # Weights + bias on the scalar-engine HWDGE ring so the big store
    # stream on the sync ring is never queued behind them.
    wcat = cpool.tile([D, FREE], MM_DT, tag="wcat")
    for e in range(EPC):
        nc.scalar.dma_start(wcat[:, e * D : (e + 1) * D], w_d[e])
    bcat = cpool.tile([128, FREE], F32, tag="bcat")
    nc.scalar.dma_start(bcat[:], b_d[:])

    xch = []
    for c in range(NCHUNK):
        t = xpool.tile([D, CH], MM_DT, tag=f"xc{c}")
        nc.scalar.dma_start(t[:], xT_d[:, c * CH : (c + 1) * CH])
        xch.append(t)

    for tb in range(NB):
        c, off = divmod(tb * 128, CH)
        xblk = xch[c][:, off : off + 128]
        st = spool.tile([128, FREE], F32, tag="st")
        for h in range(FREE // MM_N):
            sl = slice(h * MM_N, (h + 1) * MM_N)
            ps = ppool.tile([128, MM_N], F32, tag="ps")
            nc.tensor.matmul(ps[:], lhsT=xblk, rhs=wcat[:, sl], start=True, stop=True)
            nc.vector.tensor_add(st[:, sl], ps[:], bcat[:, sl])
        nc.sync.dma_start(out_v[tb], st[:])


def _build(repeats=1):
    key = (MM_DT, repeats)
    if key in _built:
        return _built[key]

    nc = bacc.Bacc("TRN2", debug=False, num_devices=M)
    xT_d = nc.dram_tensor("xT", [D, N], MM_DT, kind="ExternalInput").ap()
    w_d = nc.dram_tensor("w", [EPC, D, D], MM_DT, kind="ExternalInput").ap()
    b_d = nc.dram_tensor("bbc", [128, FREE], F32, kind="ExternalInput").ap()
    out_d = nc.dram_tensor("out", [N, EPC, D], F32, kind="ExternalOutput").ap()
    out_v = out_d.rearrange("(nb p) e o -> nb p (e o)", p=128)

    ET = mybir.EngineType
    with tile.TileContext(nc) as tc:
        with ExitStack() as ctx:
            if repeats == 1:
                _body(nc, xT_d, w_d, b_d, out_v, ctx, tc)
            else:
                with tc.For_i(
                    0, repeats, 1, hint_engines=(ET.PE, ET.DVE, ET.SP, ET.Activation)
                ):
                    _body(nc, xT_d, w_d, b_d, out_v, ctx, tc)

    nc.compile()
    _built[key] = nc
    return nc


def _in_maps(inputs, W, b):
    x = np.ascontiguousarray(np.asarray(inputs, dtype=np.float32)[:, 0, :])
    xT = np.ascontiguousarray(x.T)
    W = np.asarray(W, dtype=np.float32)
    b = np.asarray(b, dtype=np.float32)
    maps = []
    for m in range(M):
        wm = np.ascontiguousarray(W[m * EPC : (m + 1) * EPC])
        bbc = np.broadcast_to(
            b[m * EPC : (m + 1) * EPC].reshape(1, FREE), (128, FREE)
        ).copy()
        maps.append({"xT": xT, "w": wm, "bbc": bbc})
    return maps


def kernel(inputs, W, b, _repeats=1, _result_sink=None):
    nc = _build(_repeats)
    res = run_bass_kernel_spmd(nc, _in_maps(inputs, W, b), core_ids=list(range(M)))
    if _result_sink is not None:
        _result_sink.append(res)
    return np.concatenate([res.results[m]["out"] for m in range(M)], axis=1)


# revision 30
# speedup vs baseline: 256.6465x; 256.6465x over previous
"""Trainium2 Bass kernel for grouped expert GEMM (MoE forward).

Computes out[n, e, d] = sum_k x[n, k] * W[e, k, d] + b[e, d] for
N=16384 tokens, E=64 experts, D=128, fp32.

Expert-parallel across 8 NeuronCores (no cross-device communication):
core m owns experts [8m, 8m+8), reads the full token matrix, and
produces out[:, 8m:8m+8, :]; the host concatenates along dim 1.

Per-core kernel (memory-bound; ~64.5 MB stores + ~9 MB loads per core,
measured ~230 us/core ~= the per-core HBM streaming floor):
  - host pre-transposes x -> xT [D=128, N] so the contraction dim (D)
    lands on SBUF partitions; xT is DMA'd to SBUF in 8 x 1MB chunks.
  - the 8 owned experts' weights are concatenated along the free dim in
    SBUF: wcat [128, 8*128]. For each 128-token block, the token block
    xT[:, t:t+128] is the stationary matmul operand and wcat streams
    through in two 512-wide fp32r matmuls -> psum[t, (e,d)].
    (float32r streams 1 col/cycle vs 4 for plain float32, at TF32-like
    precision: whole-output relative error ~1.5e-4.)
  - bias (host-broadcast to [128, 8*128]) is fused into the PSUM->SBUF
    drain as a vector-engine tensor_add.
  - each token block's staged [128, 1024] result is one contiguous
    512 KB DMA store; input loads ride the scalar-engine HWDGE ring so
    stores on the sync ring are never queued behind them.
"""

import os
import sys

if not any("trn_rl_repo" in p for p in sys.path):
    sys.path.insert(0, "/opt/trn_rl_repo")

from contextlib import ExitStack

import numpy as np

import concourse.bacc as bacc
import concourse.tile as tile
from concourse import mybir
from concourse.bass_utils import run_bass_kernel_spmd

N, E, D = 16384, 64, 128
M = 8                 # cores
EPC = E // M          # experts per core
FREE = EPC * D        # concatenated expert free dim = 1024
MM_N = 512            # max fp32 matmul free dim (one PSUM bank)
NB = N // 128         # 128-token blocks
NCHUNK = 8            # xT load chunks
CH = N // NCHUNK

F32 = mybir.dt.float32
F32R = mybir.dt.float32r

_built = {}


def _body(nc, xT_d, w_d, b_d, out_v, ctx, tc):
    xpool = ctx.enter_context(tc.tile_pool(name="x", bufs=1))
    cpool = ctx.enter_context(tc.tile_pool(name="const", bufs=1))
    spool = ctx.enter_context(tc.tile_pool(name="stage", bufs=6))
    ppool = ctx.enter_context(tc.tile_pool(name="psum", bufs=8, space="PSUM"))

    wcat = cpool.tile([D, FREE], F32R, tag="wcat")
    for e in range(EPC):
        nc.scalar.dma_start(wcat[:, e * D : (e + 1) * D], w_d[e])
    bcat = cpool.tile([128, FREE], F32, tag="bcat")
    nc.scalar.dma_start(bcat[:], b_d[:])

    xch = []
    for c in range(NCHUNK):
        t = xpool.tile([D, CH], F32R, tag=f"xc{c}")
        nc.scalar.dma_start(t[:], xT_d[:, c * CH : (c + 1) * CH])
        xch.append(t)

    for tb in range(NB):
        c, off = divmod(tb * 128, CH)
        xblk = xch[c][:, off : off + 128]
        st = spool.tile([128, FREE], F32, tag="st")
        for h in range(FREE // MM_N):
            sl = slice(h * MM_N, (h + 1) * MM_N)
            ps = ppool.tile([128, MM_N], F32, tag="ps")
            nc.tensor.matmul(ps[:], lhsT=xblk, rhs=wcat[:, sl], start=True, stop=True)
            nc.vector.tensor_add(st[:, sl], ps[:], bcat[:, sl])
        nc.sync.dma_start(out_v[tb], st[:])


def _build(repeats=1, internal_out=False):
    """repeats>1 wraps the body in a hardware loop (timing harness only);
    internal_out stores to DRAM scratch with a tiny external output so a
    timing call's D2H is negligible."""
    key = (repeats, internal_out)
    if key in _built:
        return _built[key]

    nc = bacc.Bacc("TRN2", debug=False, num_devices=M)
    xT_d = nc.dram_tensor("xT", [D, N], F32R, kind="ExternalInput").ap()
    w_d = nc.dram_tensor("w", [EPC, D, D], F32R, kind="ExternalInput").ap()
    b_d = nc.dram_tensor("bbc", [128, FREE], F32, kind="ExternalInput").ap()
    if internal_out:
        out_d = nc.dram_tensor("scratch", [N, EPC, D], F32).ap()
        tiny = nc.dram_tensor("out", [1, 1], F32, kind="ExternalOutput").ap()
    else:
        out_d = nc.dram_tensor("out", [N, EPC, D], F32, kind="ExternalOutput").ap()
        tiny = None
    out_v = out_d.rearrange("(nb p) e o -> nb p (e o)", p=128)

    ET = mybir.EngineType
    with tile.TileContext(nc) as tc:
        with ExitStack() as ctx:
            if repeats == 1:
                _body(nc, xT_d, w_d, b_d, out_v, ctx, tc)
            else:
                with tc.For_i(
                    0, repeats, 1, hint_engines=(ET.PE, ET.DVE, ET.SP, ET.Activation)
                ):
                    _body(nc, xT_d, w_d, b_d, out_v, ctx, tc)
            if tiny is not None:
                tpool = ctx.enter_context(tc.tile_pool(name="tiny", bufs=1))
                tt = tpool.tile([1, 1], F32)
                nc.vector.memset(tt[:], 0.0)
                nc.sync.dma_start(tiny[:], tt[:])

    nc.compile()
    _built[key] = nc
    return nc


def _in_maps(inputs, W, b):
    x = np.ascontiguousarray(np.asarray(inputs, dtype=np.float32)[:, 0, :])
    xT = np.ascontiguousarray(x.T)
    W = np.asarray(W, dtype=np.float32)
    b = np.asarray(b, dtype=np.float32)
    maps = []
    for m in range(M):
        wm = np.ascontiguousarray(W[m * EPC : (m + 1) * EPC])
        bbc = np.broadcast_to(
            b[m * EPC : (m + 1) * EPC].reshape(1, FREE), (128, FREE)
        ).copy()
        maps.append({"xT": xT, "w": wm, "bbc": bbc})
    return maps


def kernel(inputs, W, b):
    nc = _build()
    res = run_bass_kernel_spmd(nc, _in_maps(inputs, W, b), core_ids=list(range(M)))
    return np.concatenate([res.results[m]["out"] for m in range(M)], axis=1)


# revision 31
# speedup vs baseline: 281.0483x; 1.0951x over previous
"""Trainium2 Bass kernel for grouped expert GEMM (MoE forward).

Computes out[n, e, d] = sum_k x[n, k] * W[e, k, d] + b[e, d] for
N=16384 tokens, E=64 experts, D=128, fp32.

Hybrid sharding across 8 NeuronCores, 2-way experts x 4-way tokens
(no cross-device communication; host scatters inputs / gathers output).

Core m = (me, mt) with me = m//4, mt = m%4 owns experts [32*me, 32*me+32)
and tokens [4096*mt, 4096*mt+4096): reads x-shard 2MB + W-half 2MB + bias
row 16KB (vs 8.5MB expert-parallel), writes the same 64MB.

Per-block structure is identical to the expert-parallel kernel (stationary
128-token block, two 512-wide f32r matmuls per expert-group of 8, DVE
bias-add fused into the PSUM drain, 512KB stores) -- stores are strided
(4KB rows @ 16KB stride), measured at full DMA rate. Bias is broadcast
across partitions on-chip once via K=1 matmuls.
"""

import os
import sys

if not any("trn_rl_repo" in p for p in sys.path):
    sys.path.insert(0, "/opt/trn_rl_repo")

from contextlib import ExitStack

import numpy as np

import concourse.bacc as bacc
import concourse.tile as tile
from concourse import mybir
from concourse.bass_utils import run_bass_kernel_spmd

N, E, D = 16384, 64, 128
M = 8
ESPLIT, TSPLIT = 2, 4
EPC = E // ESPLIT     # 32 experts per core
TPC = N // TSPLIT     # 4096 tokens per core
FREEC = EPC * D       # 4096 free columns per core
EG = 8                # experts per inner group
GFREE = EG * D        # 1024 free columns per group
NG = EPC // EG        # 4 groups
MM_N = 512

F32 = mybir.dt.float32
F32R = mybir.dt.float32r

_built = {}


def _body(nc, xT_d, w_d, b1_d, ones_d, out_v, ctx, tc):
    cpool = ctx.enter_context(tc.tile_pool(name="const", bufs=1))
    sbufs = int(os.environ.get("KERNEL_STAGE_BUFS", "6"))
    pbufs = int(os.environ.get("KERNEL_PSUM_BUFS", "8"))
    spool = ctx.enter_context(tc.tile_pool(name="stage", bufs=sbufs))
    ppool = ctx.enter_context(tc.tile_pool(name="psum", bufs=pbufs, space="PSUM"))

    wcat = cpool.tile([D, FREEC], F32R, tag="wcat")
    for e in range(EPC):
        nc.scalar.dma_start(wcat[:, e * D : (e + 1) * D], w_d[e])
    b1 = cpool.tile([1, FREEC], F32R, tag="b1")
    nc.scalar.dma_start(b1[:], b1_d[:])
    ones = cpool.tile([1, 128], F32R, tag="ones")
    nc.scalar.dma_start(ones[:], ones_d[:])
    xt = cpool.tile([D, TPC], F32R, tag="xt")
    nc.scalar.dma_start(xt[:], xT_d[:])

    # On-chip bias broadcast: bcat[p, c] = b1[c].
    bcat = cpool.tile([128, FREEC], F32, tag="bcat")
    for q in range(FREEC // MM_N):
        sl = slice(q * MM_N, (q + 1) * MM_N)
        bp = ppool.tile([128, MM_N], F32, tag="ps")
        nc.tensor.matmul(bp[:], lhsT=ones[:], rhs=b1[:, sl], start=True, stop=True)
        nc.vector.tensor_copy(bcat[:, sl], bp[:])

    for tb in range(TPC // 128):
        xblk = xt[:, tb * 128 : (tb + 1) * 128]
        for eg in range(NG):
            st = spool.tile([128, GFREE], F32, tag="st")
            for h in range(GFREE // MM_N):
                sl = slice(eg * GFREE + h * MM_N, eg * GFREE + (h + 1) * MM_N)
                ps = ppool.tile([128, MM_N], F32, tag="ps")
                nc.tensor.matmul(
                    ps[:], lhsT=xblk, rhs=wcat[:, sl], start=True, stop=True
                )
                nc.vector.tensor_add(
                    st[:, h * MM_N : (h + 1) * MM_N], ps[:], bcat[:, sl]
                )
            nc.sync.dma_start(
                out_v[tb][:, eg * GFREE : (eg + 1) * GFREE], st[:]
            )


def _build(repeats=1, internal_out=False):
    key = (repeats, internal_out)
    if key in _built:
        return _built[key]
    nc = bacc.Bacc("TRN2", debug=False, num_devices=M)
    xT_d = nc.dram_tensor("xTq", [D, TPC], F32R, kind="ExternalInput").ap()
    w_d = nc.dram_tensor("w", [EPC, D, D], F32R, kind="ExternalInput").ap()
    b1_d = nc.dram_tensor("b1h", [1, FREEC], F32R, kind="ExternalInput").ap()
    ones_d = nc.dram_tensor("onesv", [1, 128], F32R, kind="ExternalInput").ap()
    if internal_out:
        out_d = nc.dram_tensor("scratch", [TPC, EPC, D], F32).ap()
        tiny = nc.dram_tensor("out", [1, 1], F32, kind="ExternalOutput").ap()
    else:
        out_d = nc.dram_tensor("out", [TPC, EPC, D], F32, kind="ExternalOutput").ap()
        tiny = None
    out_v = out_d.rearrange("(nb p) e o -> nb p (e o)", p=128)

    ET = mybir.EngineType
    with tile.TileContext(nc) as tc:
        with ExitStack() as ctx:
            if repeats == 1:
                _body(nc, xT_d, w_d, b1_d, ones_d, out_v, ctx, tc)
            else:
                with tc.For_i(
                    0, repeats, 1, hint_engines=(ET.PE, ET.DVE, ET.SP, ET.Activation)
                ):
                    _body(nc, xT_d, w_d, b1_d, ones_d, out_v, ctx, tc)
            if tiny is not None:
                tpool = ctx.enter_context(tc.tile_pool(name="tiny", bufs=1))
                tt = tpool.tile([1, 1], F32)
                nc.vector.memset(tt[:], 0.0)
                nc.sync.dma_start(tiny[:], tt[:])
    nc.compile()
    _built[key] = nc
    return nc


def _in_maps(inputs, W, b):
    x = np.ascontiguousarray(np.asarray(inputs, dtype=np.float32)[:, 0, :])
    xT = np.ascontiguousarray(x.T)
    W = np.asarray(W, dtype=np.float32)
    b = np.asarray(b, dtype=np.float32)
    onesv = np.ones((1, 128), dtype=np.float32)
    maps = []
    for m in range(M):
        me, mt = divmod(m, TSPLIT)
        maps.append(
            {
                "xTq": np.ascontiguousarray(xT[:, mt * TPC : (mt + 1) * TPC]),
                "w": np.ascontiguousarray(W[me * EPC : (me + 1) * EPC]),
                "b1h": np.ascontiguousarray(
                    b[me * EPC : (me + 1) * EPC].reshape(1, FREEC)
                ),
                "onesv": onesv,
            }
        )
    return maps


def kernel(inputs, W, b):
    nc = _build()
    res = run_bass_kernel_spmd(nc, _in_maps(inputs, W, b), core_ids=list(range(M)))
    full = np.empty((N, E, D), dtype=np.float32)
    for m in range(M):
        me, mt = divmod(m, TSPLIT)
        full[mt * TPC : (mt + 1) * TPC, me * EPC : (me + 1) * EPC, :] = res.results[
            m
        ]["out"]
    return full
